# revision 24
# baseline (speedup 1.0000x reference)
"""Cross-modal center contrastive loss on 8 Trainium2 NeuronCores.

Math: every entry of the reference's 4096x4096 distance matrix depends only on
the *class pair* (targets[i], targets[j]), because centersR[i] = class_mean[t_i]
and centers[i] = centers_param[t_i].  The loss collapses to a C x C
computation weighted by class counts:

    loss = (1/N^2) * sum_m [  sum_a cnt_a^2 * sq_m[a, a]
                            + sum_{a != b} cnt_a * cnt_b * relu(0.5 - d_m[a, b])^2 ]

with sq_m[a, b] = clip(||mean_m[a] - centers_param[b]||^2, 1e-12), d = sqrt(sq).

Device plan (SPMD over 8 cores, no collectives - the ncfw first-collective
barrier plus three serialized collective ops cost ~70us on this part, far
more than re-reading the features):
  cores 0-3 own modality 1, cores 4-7 own modality 2; core k owns the
  100-class chunk ck = k%4 (classes padded 395->400).
  Each core streams the FULL feature matrix of its modality (f16-cast on
  host, 4 MB) and accumulates, directly in PSUM:
    sums_own [100, 512]  one-hot(own-chunk) matmul against features
    cnt_cat  [1, 500]    ones-matmul against [full | own-chunk] one-hot
  The one-hot is ONE f16 DVE compare per 128-row tile against a host-built
  [iota(400) | own-class iota(100)] index tile, so the same tile feeds both
  matmuls.  Phase 2 computes the 100 x 400 distance block (means via
  inverse-count scaling, -2*dot on PE with transposed means, center norms
  folded in via a K=1 matmul), margin weighting, and a fused count-weighted
  reduction to one scalar partial.  Host sums the 8 partials.
"""

import numpy as np

from concourse import bacc, bass, mybir
from concourse import tile
from concourse.bass_utils import run_bass_kernel_spmd

F32 = mybir.dt.float32
F32R = mybir.dt.float32r
F16 = mybir.dt.float16
U8 = mybir.dt.uint8
AT = mybir.ActivationFunctionType
OP = mybir.AluOpType
AX = mybir.AxisListType

NCORES = 8
N = 4096          # batch rows
D = 512           # feature dim
CR = 395          # real number of classes
C1 = 400          # padded class count (f32r matmul wants free dim % 4 == 0)
MCH = C1 // 4     # class chunk owned per core
CCAT = C1 + MCH   # one-hot width: [full 400 | own 100]
P = 128           # SBUF partitions
RT = N // P       # 32 row tiles (full modality per core)
GD = 1            # row tiles per feature DMA
INV_N2 = 1.0 / (N * N)  # 2^-24, exact in f32


def build_nc() -> bass.Bass:
    nc = bacc.Bacc(None, num_devices=NCORES)

    feat = nc.dram_tensor("feat", [N, D], F32, kind="ExternalInput")
    tgt = nc.dram_tensor("tgt", [N, 1], F32, kind="ExternalInput")
    rowcls = nc.dram_tensor("rowcls", [MCH, 1], F32, kind="ExternalInput")
    iocat = nc.dram_tensor("iocat", [P, CCAT], F32, kind="ExternalInput")
    cT = nc.dram_tensor("cT", [D, C1], F32, kind="ExternalInput")
    out_part = nc.dram_tensor("out_part", [1, 1], F32, kind="ExternalOutput")

    with tile.TileContext(nc, num_cores=NCORES) as tc:
        with (
            tc.tile_pool(name="const", bufs=1) as cb,
            tc.tile_pool(name="sb", bufs=1) as sb,
            tc.tile_pool(name="stream", bufs=4) as stm,
            tc.tile_pool(name="ps", bufs=1, space="PSUM") as ps,
        ):
            # ---- constants ----
            iota = cb.tile([P, C1], F32)  # iota[p, j] = j
            nc.gpsimd.iota(iota[:], pattern=[[1, C1]], base=0,
                           channel_multiplier=0,
                           allow_small_or_imprecise_dtypes=True)
            ones_col = cb.tile([P, 1], F32R)
            nc.vector.tensor_scalar(ones_col[:], iota[:, 0:1], -1.0, None,
                                    OP.is_gt)
            ones_colr = cb.tile([P, 1], F32R)
            nc.vector.tensor_scalar(ones_colr[:], iota[:, 0:1], -1.0, None,
                                    OP.is_gt)
            ones_row = cb.tile([1, P], F32R)
            nc.vector.tensor_scalar(ones_row[:], iota[0:1, 0:P], -1.0, None,
                                    OP.is_gt)
            half_col = cb.tile([P, 1], F32)
            nc.vector.memset(half_col[:], 0.5)
            ident = cb.tile([P, P], F32)
            nc.gpsimd.memset(ident[:], 0.0)
            nc.gpsimd.affine_select(out=ident[:], in_=ident[:],
                                    compare_op=OP.not_equal, fill=1.0,
                                    base=0, pattern=[[-1, P]],
                                    channel_multiplier=1)

            # warm the ACT LUTs (Square/Sqrt/Relu) while DMA streams
            warm = cb.tile([1, 1], F32)
            nc.scalar.activation(warm[:], half_col[0:1, :], AT.Square)
            nc.scalar.activation(warm[:], warm[:], AT.Sqrt)
            nc.scalar.activation(warm[:], warm[:], AT.Relu,
                                 bias=half_col[0:1, :], scale=-1.0)

            # per-core inputs
            io_sb = sb.tile([P, CCAT], F32)
            nc.sync.dma_start(io_sb[:], iocat[:])
            rc = sb.tile([MCH, 1], F32)
            nc.gpsimd.dma_start(rc[:], rowcls[:])
            tgt_sb = sb.tile([P, RT], F32)   # targets, column r = rows rP..rP+127
            nc.sync.dma_start(
                tgt_sb[:], tgt.rearrange("(r p) one -> p (r one)", r=RT))
            ct_tiles = []
            for j in range(4):
                ctj = sb.tile([P, C1], F32R, name=f"ct_{j}")
                nc.sync.dma_start(ctj[:], cT[j * P:(j + 1) * P, :].bitcast(F32R))
                ct_tiles.append(ctj)

            # diagonal mask (only needs iota + rowcls; independent of phase 1)
            mask = sb.tile([MCH, C1], U8)
            nc.vector.tensor_scalar(mask[:], iota[0:MCH, :], rc[:], None,
                                    OP.is_equal)

            # ---- phase 1: stream features, accumulate sums + counts ----
            sums_ps = ps.tile([MCH, D], F32, name="sums_ps")
            cnt_ps = ps.tile([1, CCAT], F32, name="cnt_ps")
            for i in range(RT // GD):
                fr = stm.tile([P, GD * D], F32R, name="fr", tag="fr", bufs=3)
                nc.sync.dma_start(
                    fr[:].rearrange("p (g d) -> p g d", g=GD),
                    feat[i * GD * P:(i + 1) * GD * P, :]
                    .rearrange("(g p) d -> p g d", g=GD).bitcast(F32R))
                for g in range(GD):
                    r = i * GD + g
                    oh = stm.tile([P, CCAT], F32R, name="oh", tag="oh", bufs=4)
                    nc.vector.tensor_scalar(oh[:], io_sb[:],
                                            tgt_sb[:, r:r + 1], None,
                                            OP.is_equal)
                    oh_own = stm.tile([P, MCH], F32R, name="oh_own",
                                      tag="oh_own", bufs=4)
                    nc.vector.tensor_scalar(oh_own[:], io_sb[:, C1:CCAT],
                                            tgt_sb[:, r:r + 1], None,
                                            OP.is_equal)
                    nc.tensor.matmul(sums_ps[:], oh_own[:],
                                     fr[:, g * D:(g + 1) * D],
                                     start=(r == 0), stop=(r == RT - 1))
                    nc.tensor.matmul(cnt_ps[:], ones_col[:], oh[:],
                                     start=(r == 0), stop=(r == RT - 1))

            so = sb.tile([MCH, D], F32)
            nc.vector.tensor_copy(so[:], sums_ps[:])
            ca = sb.tile([1, C1], F32)
            nc.vector.tensor_copy(ca[:], cnt_ps[0:1, 0:C1])
            cno_row = sb.tile([1, MCH], F32)
            nc.vector.tensor_copy(cno_row[:], cnt_ps[0:1, C1:CCAT])
            # own-chunk counts as a column: PE transpose of [1, 100]
            cot_ps = ps.tile([MCH, 1], F32, name="cot_ps")
            nc.tensor.transpose(cot_ps[:], cno_row[:], ident[0:1, 0:1])
            co = sb.tile([MCH, 1], F32)
            nc.vector.tensor_copy(co[:], cot_ps[:])

            # ---- phase 2: own 100-class chunk vs all 400 centers ----
            clamped = sb.tile([MCH, 1], F32)
            nc.vector.tensor_scalar(clamped[:], co[:], 1.0, None, OP.max)
            inv = sb.tile([MCH, 1], F32)
            nc.vector.reciprocal(inv[:], clamped[:])
            inv_n2 = sb.tile([MCH, 1], F32)
            nc.vector.tensor_scalar(inv_n2[:], inv[:], -2.0, None, OP.mult)
            inv2 = sb.tile([MCH, 1], F32)
            nc.vector.tensor_tensor(inv2[:], inv[:], inv[:], OP.mult)
            snorm = sb.tile([MCH, 1], F32)
            sqtmp = sb.tile([MCH, D], F32)
            nc.scalar.activation(sqtmp[:], so[:], AT.Square, accum_out=snorm[:])
            mnorm = sb.tile([MCH, 1], F32)
            nc.vector.tensor_tensor(mnorm[:], snorm[:], inv2[:], OP.mult)
            # means scaled by -2 (so the PE dot matmul directly yields -2*dot)
            means_n2 = sb.tile([MCH, D], F32)
            nc.vector.tensor_scalar(means_n2[:], so[:], inv_n2[:], None,
                                    OP.mult)

            mt_tiles = []
            for j in range(4):
                tp = ps.tile([P, MCH], F32, name="tp", tag="tp", bufs=2)
                nc.tensor.transpose(tp[:], means_n2[:, j * P:(j + 1) * P],
                                    ident[0:MCH, 0:MCH])
                mt = sb.tile([P, MCH], F32R, name=f"mt_{j}")
                nc.vector.tensor_copy(mt[:], tp[:])
                mt_tiles.append(mt)

            cn_ps = ps.tile([1, C1], F32, name="cn_ps")
            for j in range(4):
                csq = stm.tile([P, C1], F32R, name="csq", tag="csq")
                nc.vector.tensor_tensor(csq[:], ct_tiles[j][:], ct_tiles[j][:],
                                        OP.mult)
                nc.tensor.matmul(cn_ps[:], ones_colr[:], csq[:],
                                 start=(j == 0), stop=(j == 3))
            cnorm = sb.tile([1, C1], F32R)
            nc.vector.tensor_copy(cnorm[:], cn_ps[:])

            sq_ps = ps.tile([MCH, C1], F32, name="sq_ps")
            for j in range(4):
                nc.tensor.matmul(sq_ps[:], mt_tiles[j][:], ct_tiles[j][:],
                                 start=(j == 0), stop=False)
            nc.tensor.matmul(sq_ps[:], ones_row[:, 0:MCH], cnorm[:],
                             start=False, stop=True)

            # sq = max(-2*dot + cnorm + mnorm, 1e-12)
            sq_sb = sb.tile([MCH, C1], F32)
            nc.vector.tensor_scalar(sq_sb[:], sq_ps[:], mnorm[:], 1e-12,
                                    OP.add, OP.max)
            d_sb = sb.tile([MCH, C1], F32)
            nc.scalar.activation(d_sb[:], sq_sb[:], AT.Sqrt)
            neg = sb.tile([MCH, C1], F32)
            nc.scalar.activation(neg[:], d_sb[:], AT.Relu,
                                 bias=half_col[0:MCH, :], scale=-1.0)
            negsq = sb.tile([MCH, C1], F32)
            nc.vector.tensor_tensor(negsq[:], neg[:], neg[:], OP.mult)

            # T = sq on the global diagonal (column == own class), negsq off it
            tsel = sb.tile([MCH, C1], F32)
            nc.vector.select(tsel[:], mask[:], sq_sb[:], negsq[:])

            # S_partial = (1/N^2) sum_b cnt_all[b] * sum_a cnt_own[a] * T[a, b]
            cs_ps = ps.tile([1, C1], F32, name="cs_ps")
            nc.tensor.matmul(cs_ps[:], co[:], tsel[:], start=True, stop=True)
            wrow = sb.tile([1, C1], F32)
            nc.vector.tensor_tensor(wrow[:], cs_ps[:], ca[:], OP.mult)
            acc = sb.tile([1, 1], F32)
            nc.vector.tensor_reduce(acc[:], wrow[:], AX.X, OP.add)
            part = sb.tile([1, 1], F32)
            nc.vector.tensor_scalar(part[:], acc[:], INV_N2, None, OP.mult)
            nc.gpsimd.dma_start(out_part[:], part[:])

    if not nc.is_finalized():
        nc.finalize()
    return nc


_NC_CACHE = None


def _get_nc() -> bass.Bass:
    global _NC_CACHE
    if _NC_CACHE is None:
        _NC_CACHE = build_nc()
    return _NC_CACHE


def make_in_maps(modal1_inputs, modal2_inputs, centers_param, targets):
    m1 = np.ascontiguousarray(np.asarray(modal1_inputs, dtype=np.float32))
    m2 = np.ascontiguousarray(np.asarray(modal2_inputs, dtype=np.float32))
    cp = np.asarray(centers_param, dtype=np.float32)
    t = np.asarray(targets).astype(np.float32).reshape(N, 1)
    cT = np.zeros((D, C1), dtype=np.float32)
    cT[:, :CR] = cp.T
    in_maps = []
    for k in range(NCORES):
        ck = k % 4
        cls0 = ck * MCH
        iocat = np.empty((P, CCAT), dtype=np.float32)
        iocat[:, :C1] = np.arange(C1, dtype=np.float32)
        iocat[:, C1:] = np.arange(cls0, cls0 + MCH, dtype=np.float32)
        in_maps.append({
            "feat": m1 if k < 4 else m2,
            "tgt": t,
            "rowcls": np.arange(cls0, cls0 + MCH,
                                dtype=np.float32).reshape(MCH, 1),
            "iocat": iocat,
            "cT": cT,
        })
    return in_maps


def run(modal1_inputs, modal2_inputs, centers_param, targets, trace=False):
    nc = _get_nc()
    in_maps = make_in_maps(modal1_inputs, modal2_inputs, centers_param, targets)
    res = run_bass_kernel_spmd(nc, in_maps, list(range(NCORES)), trace=trace)
    parts = np.array([res.results[k]["out_part"][0, 0] for k in range(NCORES)],
                     dtype=np.float32)
    loss = np.array(parts.sum(), dtype=np.float32)
    return loss, res


def kernel(modal1_inputs, modal2_inputs, centers_param, targets):
    loss, _ = run(modal1_inputs, modal2_inputs, centers_param, targets)
    return loss


# revision 25
# speedup vs baseline: 1.0294x; 1.0294x over previous
"""Cross-modal center contrastive loss on 8 Trainium2 NeuronCores.

Math: every entry of the reference's 4096x4096 distance matrix depends only on
the *class pair* (targets[i], targets[j]), because centersR[i] = class_mean[t_i]
and centers[i] = centers_param[t_i].  The loss collapses to a C x C
computation weighted by class counts:

    loss = (1/N^2) * sum_m [  sum_a cnt_a^2 * sq_m[a, a]
                            + sum_{a != b} cnt_a * cnt_b * relu(0.5 - d_m[a, b])^2 ]

with sq_m[a, b] = clip(||mean_m[a] - centers_param[b]||^2, 1e-12), d = sqrt(sq).

Device plan (SPMD over 8 cores, no collectives - the ncfw first-collective
barrier plus three serialized collective ops cost ~70us on this part, far
more than re-reading the features):
  cores 0-3 own modality 1, cores 4-7 own modality 2; core k owns the
  100-class chunk ck = k%4 (classes padded 395->400).
  Each core streams the FULL feature matrix of its modality (f16-cast on
  host, 4 MB) and accumulates, directly in PSUM:
    sums_own [100, 512]  one-hot(own-chunk) matmul against features
    cnt_cat  [1, 500]    ones-matmul against [full | own-chunk] one-hot
  The one-hot is ONE f16 DVE compare per 128-row tile against a host-built
  [iota(400) | own-class iota(100)] index tile, so the same tile feeds both
  matmuls.  Phase 2 computes the 100 x 400 distance block (means via
  inverse-count scaling, -2*dot on PE with transposed means, center norms
  folded in via a K=1 matmul), margin weighting, and a fused count-weighted
  reduction to one scalar partial.  Host sums the 8 partials.
"""

import numpy as np

from concourse import bacc, bass, mybir
from concourse import tile
from concourse.bass_utils import run_bass_kernel_spmd

F32 = mybir.dt.float32
F32R = mybir.dt.float32r
F16 = mybir.dt.float16
U8 = mybir.dt.uint8
AT = mybir.ActivationFunctionType
OP = mybir.AluOpType
AX = mybir.AxisListType

NCORES = 8
N = 4096          # batch rows
D = 512           # feature dim
CR = 395          # real number of classes
C1 = 400          # padded class count (f32r matmul wants free dim % 4 == 0)
MCH = C1 // 4     # class chunk owned per core
CCAT = C1 + MCH   # one-hot width: [full 400 | own 100]
P = 128           # SBUF partitions
RT = N // P       # 32 row tiles (full modality per core)
GD = 1            # row tiles per feature DMA
INV_N2 = 1.0 / (N * N)  # 2^-24, exact in f32


def build_nc() -> bass.Bass:
    nc = bacc.Bacc(None, num_devices=NCORES)

    feat = nc.dram_tensor("feat", [N, D], F16, kind="ExternalInput")
    tgt = nc.dram_tensor("tgt", [N, 1], F32, kind="ExternalInput")
    rowcls = nc.dram_tensor("rowcls", [MCH, 1], F32, kind="ExternalInput")
    iocat = nc.dram_tensor("iocat", [P, CCAT], F16, kind="ExternalInput")
    cT = nc.dram_tensor("cT", [D, C1], F32, kind="ExternalInput")
    out_part = nc.dram_tensor("out_part", [1, 1], F32, kind="ExternalOutput")

    with tile.TileContext(nc, num_cores=NCORES) as tc:
        with (
            tc.tile_pool(name="const", bufs=1) as cb,
            tc.tile_pool(name="sb", bufs=1) as sb,
            tc.tile_pool(name="stream", bufs=4) as stm,
            tc.tile_pool(name="ps", bufs=1, space="PSUM") as ps,
        ):
            # ---- constants ----
            iota = cb.tile([P, C1], F32)  # iota[p, j] = j
            nc.gpsimd.iota(iota[:], pattern=[[1, C1]], base=0,
                           channel_multiplier=0,
                           allow_small_or_imprecise_dtypes=True)
            ones_col = cb.tile([P, 1], F16)
            nc.vector.tensor_scalar(ones_col[:], iota[:, 0:1], -1.0, None,
                                    OP.is_gt)
            ones_colr = cb.tile([P, 1], F32R)
            nc.vector.tensor_scalar(ones_colr[:], iota[:, 0:1], -1.0, None,
                                    OP.is_gt)
            ones_row = cb.tile([1, P], F32R)
            nc.vector.tensor_scalar(ones_row[:], iota[0:1, 0:P], -1.0, None,
                                    OP.is_gt)
            half_col = cb.tile([P, 1], F32)
            nc.vector.memset(half_col[:], 0.5)
            ident = cb.tile([P, P], F32)
            nc.gpsimd.memset(ident[:], 0.0)
            nc.gpsimd.affine_select(out=ident[:], in_=ident[:],
                                    compare_op=OP.not_equal, fill=1.0,
                                    base=0, pattern=[[-1, P]],
                                    channel_multiplier=1)

            # warm the ACT LUTs (Square/Sqrt/Relu) while DMA streams
            warm = cb.tile([1, 1], F32)
            nc.scalar.activation(warm[:], half_col[0:1, :], AT.Square)
            nc.scalar.activation(warm[:], warm[:], AT.Sqrt)
            nc.scalar.activation(warm[:], warm[:], AT.Relu,
                                 bias=half_col[0:1, :], scale=-1.0)

            # per-core inputs
            io_sb = sb.tile([P, CCAT], F16)
            nc.sync.dma_start(io_sb[:], iocat[:])
            rc = sb.tile([MCH, 1], F32)
            nc.gpsimd.dma_start(rc[:], rowcls[:])
            tgt_sb = sb.tile([P, RT], F32)   # targets, column r = rows rP..rP+127
            nc.sync.dma_start(
                tgt_sb[:], tgt.rearrange("(r p) one -> p (r one)", r=RT))
            ct_tiles = []
            for j in range(4):
                ctj = sb.tile([P, C1], F32R, name=f"ct_{j}")
                nc.sync.dma_start(ctj[:], cT[j * P:(j + 1) * P, :].bitcast(F32R))
                ct_tiles.append(ctj)

            # diagonal mask (only needs iota + rowcls; independent of phase 1)
            mask = sb.tile([MCH, C1], U8)
            nc.vector.tensor_scalar(mask[:], iota[0:MCH, :], rc[:], None,
                                    OP.is_equal)

            # ---- phase 1: stream features, accumulate sums + counts ----
            sums_ps = ps.tile([MCH, D], F32, name="sums_ps")
            cnt_ps = ps.tile([1, CCAT], F32, name="cnt_ps")
            for i in range(RT // GD):
                fr = stm.tile([P, GD * D], F16, name="fr", tag="fr", bufs=3)
                nc.sync.dma_start(
                    fr[:].rearrange("p (g d) -> p g d", g=GD),
                    feat[i * GD * P:(i + 1) * GD * P, :]
                    .rearrange("(g p) d -> p g d", g=GD))
                for g in range(GD):
                    r = i * GD + g
                    oh = stm.tile([P, CCAT], F16, name="oh", tag="oh", bufs=4)
                    nc.vector.tensor_scalar(oh[:], io_sb[:],
                                            tgt_sb[:, r:r + 1], None,
                                            OP.is_equal)
                    oh_own = stm.tile([P, MCH], F16, name="oh_own",
                                      tag="oh_own", bufs=4)
                    nc.vector.tensor_scalar(oh_own[:], io_sb[:, C1:CCAT],
                                            tgt_sb[:, r:r + 1], None,
                                            OP.is_equal)
                    nc.tensor.matmul(sums_ps[:], oh_own[:],
                                     fr[:, g * D:(g + 1) * D],
                                     start=(r == 0), stop=(r == RT - 1))
                    nc.tensor.matmul(cnt_ps[:], ones_col[:], oh[:],
                                     start=(r == 0), stop=(r == RT - 1))

            so = sb.tile([MCH, D], F32)
            nc.vector.tensor_copy(so[:], sums_ps[:])
            ca = sb.tile([1, C1], F32)
            nc.vector.tensor_copy(ca[:], cnt_ps[0:1, 0:C1])
            cno_row = sb.tile([1, MCH], F32)
            nc.vector.tensor_copy(cno_row[:], cnt_ps[0:1, C1:CCAT])
            # own-chunk counts as a column: PE transpose of [1, 100]
            cot_ps = ps.tile([MCH, 1], F32, name="cot_ps")
            nc.tensor.transpose(cot_ps[:], cno_row[:], ident[0:1, 0:1])
            co = sb.tile([MCH, 1], F32)
            nc.vector.tensor_copy(co[:], cot_ps[:])

            # ---- phase 2: own 100-class chunk vs all 400 centers ----
            clamped = sb.tile([MCH, 1], F32)
            nc.vector.tensor_scalar(clamped[:], co[:], 1.0, None, OP.max)
            inv = sb.tile([MCH, 1], F32)
            nc.vector.reciprocal(inv[:], clamped[:])
            inv_n2 = sb.tile([MCH, 1], F32)
            nc.vector.tensor_scalar(inv_n2[:], inv[:], -2.0, None, OP.mult)
            inv2 = sb.tile([MCH, 1], F32)
            nc.vector.tensor_tensor(inv2[:], inv[:], inv[:], OP.mult)
            snorm = sb.tile([MCH, 1], F32)
            sqtmp = sb.tile([MCH, D], F32)
            nc.scalar.activation(sqtmp[:], so[:], AT.Square, accum_out=snorm[:])
            mnorm = sb.tile([MCH, 1], F32)
            nc.vector.tensor_tensor(mnorm[:], snorm[:], inv2[:], OP.mult)
            # means scaled by -2 (so the PE dot matmul directly yields -2*dot)
            means_n2 = sb.tile([MCH, D], F32)
            nc.vector.tensor_scalar(means_n2[:], so[:], inv_n2[:], None,
                                    OP.mult)

            mt_tiles = []
            for j in range(4):
                tp = ps.tile([P, MCH], F32, name="tp", tag="tp", bufs=2)
                nc.tensor.transpose(tp[:], means_n2[:, j * P:(j + 1) * P],
                                    ident[0:MCH, 0:MCH])
                mt = sb.tile([P, MCH], F32R, name=f"mt_{j}")
                nc.vector.tensor_copy(mt[:], tp[:])
                mt_tiles.append(mt)

            cn_ps = ps.tile([1, C1], F32, name="cn_ps")
            for j in range(4):
                csq = stm.tile([P, C1], F32R, name="csq", tag="csq")
                nc.vector.tensor_tensor(csq[:], ct_tiles[j][:], ct_tiles[j][:],
                                        OP.mult)
                nc.tensor.matmul(cn_ps[:], ones_colr[:], csq[:],
                                 start=(j == 0), stop=(j == 3))
            cnorm = sb.tile([1, C1], F32R)
            nc.vector.tensor_copy(cnorm[:], cn_ps[:])

            sq_ps = ps.tile([MCH, C1], F32, name="sq_ps")
            for j in range(4):
                nc.tensor.matmul(sq_ps[:], mt_tiles[j][:], ct_tiles[j][:],
                                 start=(j == 0), stop=False)
            nc.tensor.matmul(sq_ps[:], ones_row[:, 0:MCH], cnorm[:],
                             start=False, stop=True)

            # sq = max(-2*dot + cnorm + mnorm, 1e-12)
            sq_sb = sb.tile([MCH, C1], F32)
            nc.vector.tensor_scalar(sq_sb[:], sq_ps[:], mnorm[:], 1e-12,
                                    OP.add, OP.max)
            d_sb = sb.tile([MCH, C1], F32)
            nc.scalar.activation(d_sb[:], sq_sb[:], AT.Sqrt)
            neg = sb.tile([MCH, C1], F32)
            nc.scalar.activation(neg[:], d_sb[:], AT.Relu,
                                 bias=half_col[0:MCH, :], scale=-1.0)
            negsq = sb.tile([MCH, C1], F32)
            nc.vector.tensor_tensor(negsq[:], neg[:], neg[:], OP.mult)

            # T = sq on the global diagonal (column == own class), negsq off it
            tsel = sb.tile([MCH, C1], F32)
            nc.vector.select(tsel[:], mask[:], sq_sb[:], negsq[:])

            # S_partial = (1/N^2) sum_b cnt_all[b] * sum_a cnt_own[a] * T[a, b]
            cs_ps = ps.tile([1, C1], F32, name="cs_ps")
            nc.tensor.matmul(cs_ps[:], co[:], tsel[:], start=True, stop=True)
            wrow = sb.tile([1, C1], F32)
            nc.vector.tensor_tensor(wrow[:], cs_ps[:], ca[:], OP.mult)
            acc = sb.tile([1, 1], F32)
            nc.vector.tensor_reduce(acc[:], wrow[:], AX.X, OP.add)
            part = sb.tile([1, 1], F32)
            nc.vector.tensor_scalar(part[:], acc[:], INV_N2, None, OP.mult)
            nc.gpsimd.dma_start(out_part[:], part[:])

    if not nc.is_finalized():
        nc.finalize()
    return nc


_NC_CACHE = None


def _get_nc() -> bass.Bass:
    global _NC_CACHE
    if _NC_CACHE is None:
        _NC_CACHE = build_nc()
    return _NC_CACHE


def make_in_maps(modal1_inputs, modal2_inputs, centers_param, targets):
    m1 = np.asarray(modal1_inputs, dtype=np.float32).astype(np.float16)
    m2 = np.asarray(modal2_inputs, dtype=np.float32).astype(np.float16)
    cp = np.asarray(centers_param, dtype=np.float32)
    t = np.asarray(targets).astype(np.float32).reshape(N, 1)
    cT = np.zeros((D, C1), dtype=np.float32)
    cT[:, :CR] = cp.T
    in_maps = []
    for k in range(NCORES):
        ck = k % 4
        cls0 = ck * MCH
        iocat = np.empty((P, CCAT), dtype=np.float16)
        iocat[:, :C1] = np.arange(C1, dtype=np.float16)
        iocat[:, C1:] = np.arange(cls0, cls0 + MCH, dtype=np.float16)
        in_maps.append({
            "feat": m1 if k < 4 else m2,
            "tgt": t,
            "rowcls": np.arange(cls0, cls0 + MCH,
                                dtype=np.float32).reshape(MCH, 1),
            "iocat": iocat,
            "cT": cT,
        })
    return in_maps


def run(modal1_inputs, modal2_inputs, centers_param, targets, trace=False):
    nc = _get_nc()
    in_maps = make_in_maps(modal1_inputs, modal2_inputs, centers_param, targets)
    res = run_bass_kernel_spmd(nc, in_maps, list(range(NCORES)), trace=trace)
    parts = np.array([res.results[k]["out_part"][0, 0] for k in range(NCORES)],
                     dtype=np.float32)
    loss = np.array(parts.sum(), dtype=np.float32)
    return loss, res


def kernel(modal1_inputs, modal2_inputs, centers_param, targets):
    loss, _ = run(modal1_inputs, modal2_inputs, centers_param, targets)
    return loss


# revision 26
# speedup vs baseline: 1.3856x; 1.3459x over previous
"""Cross-modal center contrastive loss on 8 Trainium2 NeuronCores.

Math: every entry of the reference's 4096x4096 distance matrix depends only on
the *class pair* (targets[i], targets[j]), because centersR[i] = class_mean[t_i]
and centers[i] = centers_param[t_i].  The loss collapses to a C x C
computation weighted by class counts:

    loss = (1/N^2) * sum_m [  sum_a cnt_a^2 * sq_m[a, a]
                            + sum_{a != b} cnt_a * cnt_b * relu(0.5 - d_m[a, b])^2 ]

with sq_m[a, b] = clip(||mean_m[a] - centers_param[b]||^2, 1e-12), d = sqrt(sq).

Device plan (SPMD over 8 cores, no collectives - the ncfw first-collective
barrier plus three serialized collective ops cost ~70us on this part, far
more than re-reading the features):
  cores 0-3 own modality 1, cores 4-7 own modality 2; core k owns the
  100-class chunk ck = k%4 (classes padded 395->400).
  Each core streams the FULL feature matrix of its modality (f16-cast on
  host, 4 MB) and accumulates, directly in PSUM:
    sums_own [100, 512]  one-hot(own-chunk) matmul against features
    cnt_cat  [1, 500]    ones-matmul against [full | own-chunk] one-hot
  The one-hot is ONE f16 DVE compare per 128-row tile against a host-built
  [iota(400) | own-class iota(100)] index tile, so the same tile feeds both
  matmuls.  Phase 2 computes the 100 x 400 distance block (means via
  inverse-count scaling, -2*dot on PE with transposed means, center norms
  folded in via a K=1 matmul), margin weighting, and a fused count-weighted
  reduction to one scalar partial.  Host sums the 8 partials.
"""

import numpy as np

from concourse import bacc, bass, mybir
from concourse import tile
from concourse.bass_utils import run_bass_kernel_spmd

F32 = mybir.dt.float32
F32R = mybir.dt.float32r
F16 = mybir.dt.float16
U8 = mybir.dt.uint8
AT = mybir.ActivationFunctionType
OP = mybir.AluOpType
AX = mybir.AxisListType

NCORES = 8
N = 4096          # batch rows
D = 512           # feature dim
CR = 395          # real number of classes
C1 = 400          # padded class count (f32r matmul wants free dim % 4 == 0)
MCH = C1 // 4     # class chunk owned per core
CCAT = 512        # one-hot width: [own 100 | full 400 | 12 pad]
P = 128           # SBUF partitions
RT = N // P       # 32 row tiles (full modality per core)
GD = 4            # row tiles per feature DMA
INV_N2 = 1.0 / (N * N)  # 2^-24, exact in f32


def build_nc() -> bass.Bass:
    nc = bacc.Bacc(None, num_devices=NCORES)

    feat = nc.dram_tensor("feat", [N, D], F16, kind="ExternalInput")
    tgt = nc.dram_tensor("tgt", [N, 1], F32, kind="ExternalInput")
    rowcls = nc.dram_tensor("rowcls", [MCH, 1], F32, kind="ExternalInput")
    iocat = nc.dram_tensor("iocat", [P, CCAT], F16, kind="ExternalInput")
    cT = nc.dram_tensor("cT", [D, C1], F32, kind="ExternalInput")
    out_part = nc.dram_tensor("out_part", [1, 1], F32, kind="ExternalOutput")

    with tile.TileContext(nc, num_cores=NCORES) as tc:
        with (
            tc.tile_pool(name="const", bufs=1) as cb,
            tc.tile_pool(name="sb", bufs=1) as sb,
            tc.tile_pool(name="stream", bufs=4) as stm,
            tc.tile_pool(name="ps", bufs=1, space="PSUM") as ps,
        ):
            # ---- constants ----
            iota = cb.tile([P, C1], F32)  # iota[p, j] = j
            nc.gpsimd.iota(iota[:], pattern=[[1, C1]], base=0,
                           channel_multiplier=0,
                           allow_small_or_imprecise_dtypes=True)
            ones_col = cb.tile([P, 1], F16)
            nc.vector.tensor_scalar(ones_col[:], iota[:, 0:1], -1.0, None,
                                    OP.is_gt)
            ones_colr = cb.tile([P, 1], F32R)
            nc.vector.tensor_scalar(ones_colr[:], iota[:, 0:1], -1.0, None,
                                    OP.is_gt)
            ones_row = cb.tile([1, P], F32R)
            nc.vector.tensor_scalar(ones_row[:], iota[0:1, 0:P], -1.0, None,
                                    OP.is_gt)
            half_col = cb.tile([P, 1], F32)
            nc.vector.memset(half_col[:], 0.5)
            ident = cb.tile([P, P], F32)
            nc.gpsimd.memset(ident[:], 0.0)
            nc.gpsimd.affine_select(out=ident[:], in_=ident[:],
                                    compare_op=OP.not_equal, fill=1.0,
                                    base=0, pattern=[[-1, P]],
                                    channel_multiplier=1)

            # warm the ACT LUTs (Square/Sqrt/Relu) while DMA streams
            warm = cb.tile([1, 1], F32)
            nc.scalar.activation(warm[:], half_col[0:1, :], AT.Square)
            nc.scalar.activation(warm[:], warm[:], AT.Sqrt)
            nc.scalar.activation(warm[:], warm[:], AT.Relu,
                                 bias=half_col[0:1, :], scale=-1.0)

            # per-core inputs
            io_sb = sb.tile([P, CCAT], F16)
            nc.gpsimd.dma_start(io_sb[:], iocat[:])
            rc = sb.tile([MCH, 1], F32)
            nc.gpsimd.dma_start(rc[:], rowcls[:])
            tgt_sb = sb.tile([P, RT], F32)   # targets, column r = rows rP..rP+127
            nc.sync.dma_start(
                tgt_sb[:], tgt.rearrange("(r p) one -> p (r one)", r=RT))
            ct_tiles = []
            for j in range(4):
                ctj = sb.tile([P, C1], F32R, name=f"ct_{j}")
                nc.gpsimd.dma_start(ctj[:], cT[j * P:(j + 1) * P, :].bitcast(F32R))
                ct_tiles.append(ctj)

            # diagonal mask (only needs iota + rowcls; independent of phase 1)
            mask = sb.tile([MCH, C1], U8)
            nc.vector.tensor_scalar(mask[:], iota[0:MCH, :], rc[:], None,
                                    OP.is_equal)

            # ---- phase 1: stream features, accumulate sums + counts ----
            sums_ps = ps.tile([MCH, D], F32, name="sums_ps")
            cnt_ps = ps.tile([1, CCAT], F32, name="cnt_ps")
            for i in range(RT // GD):
                fr = stm.tile([P, GD * D], F16, name="fr", tag="fr", bufs=3)
                nc.sync.dma_start(
                    fr[:].rearrange("p (g d) -> p g d", g=GD),
                    feat[i * GD * P:(i + 1) * GD * P, :]
                    .rearrange("(g p) d -> p g d", g=GD))
                for g in range(GD):
                    r = i * GD + g
                    oh = stm.tile([P, CCAT], F16, name="oh", tag="oh", bufs=4)
                    nc.vector.tensor_scalar(oh[:], io_sb[:],
                                            tgt_sb[:, r:r + 1], None,
                                            OP.is_equal)
                    nc.tensor.matmul(sums_ps[:], oh[:, 0:MCH],
                                     fr[:, g * D:(g + 1) * D],
                                     start=(r == 0), stop=(r == RT - 1))
                    nc.tensor.matmul(cnt_ps[:], ones_col[:], oh[:],
                                     start=(r == 0), stop=(r == RT - 1))

            so = sb.tile([MCH, D], F32)
            nc.vector.tensor_copy(so[:], sums_ps[:])
            ca = sb.tile([1, C1], F32)
            nc.vector.tensor_copy(ca[:], cnt_ps[0:1, MCH:MCH + C1])
            cno_row = sb.tile([1, MCH], F32)
            nc.vector.tensor_copy(cno_row[:], cnt_ps[0:1, 0:MCH])
            # own-chunk counts as a column: PE transpose of [1, 100]
            cot_ps = ps.tile([MCH, 1], F32, name="cot_ps")
            nc.tensor.transpose(cot_ps[:], cno_row[:], ident[0:1, 0:1])
            co = sb.tile([MCH, 1], F32)
            nc.vector.tensor_copy(co[:], cot_ps[:])

            # ---- phase 2: own 100-class chunk vs all 400 centers ----
            clamped = sb.tile([MCH, 1], F32)
            nc.vector.tensor_scalar(clamped[:], co[:], 1.0, None, OP.max)
            inv = sb.tile([MCH, 1], F32)
            nc.vector.reciprocal(inv[:], clamped[:])
            inv_n2 = sb.tile([MCH, 1], F32)
            nc.vector.tensor_scalar(inv_n2[:], inv[:], -2.0, None, OP.mult)
            inv2 = sb.tile([MCH, 1], F32)
            nc.vector.tensor_tensor(inv2[:], inv[:], inv[:], OP.mult)
            snorm = sb.tile([MCH, 1], F32)
            sqtmp = sb.tile([MCH, D], F32)
            nc.scalar.activation(sqtmp[:], so[:], AT.Square, accum_out=snorm[:])
            mnorm = sb.tile([MCH, 1], F32)
            nc.vector.tensor_tensor(mnorm[:], snorm[:], inv2[:], OP.mult)
            # means scaled by -2 (so the PE dot matmul directly yields -2*dot)
            means_n2 = sb.tile([MCH, D], F32)
            nc.vector.tensor_scalar(means_n2[:], so[:], inv_n2[:], None,
                                    OP.mult)

            mt_tiles = []
            for j in range(4):
                tp = ps.tile([P, MCH], F32, name="tp", tag="tp", bufs=2)
                nc.tensor.transpose(tp[:], means_n2[:, j * P:(j + 1) * P],
                                    ident[0:MCH, 0:MCH])
                mt = sb.tile([P, MCH], F32R, name=f"mt_{j}")
                nc.vector.tensor_copy(mt[:], tp[:])
                mt_tiles.append(mt)

            cn_ps = ps.tile([1, C1], F32, name="cn_ps")
            for j in range(4):
                csq = stm.tile([P, C1], F32R, name="csq", tag="csq")
                nc.vector.tensor_tensor(csq[:], ct_tiles[j][:], ct_tiles[j][:],
                                        OP.mult)
                nc.tensor.matmul(cn_ps[:], ones_colr[:], csq[:],
                                 start=(j == 0), stop=(j == 3))
            cnorm = sb.tile([1, C1], F32R)
            nc.vector.tensor_copy(cnorm[:], cn_ps[:])

            sq_ps = ps.tile([MCH, C1], F32, name="sq_ps")
            for j in range(4):
                nc.tensor.matmul(sq_ps[:], mt_tiles[j][:], ct_tiles[j][:],
                                 start=(j == 0), stop=False)
            nc.tensor.matmul(sq_ps[:], ones_row[:, 0:MCH], cnorm[:],
                             start=False, stop=True)

            # sq = max(-2*dot + cnorm + mnorm, 1e-12)
            sq_sb = sb.tile([MCH, C1], F32)
            nc.vector.tensor_scalar(sq_sb[:], sq_ps[:], mnorm[:], 1e-12,
                                    OP.add, OP.max)
            d_sb = sb.tile([MCH, C1], F32)
            nc.scalar.activation(d_sb[:], sq_sb[:], AT.Sqrt)
            neg = sb.tile([MCH, C1], F32)
            nc.scalar.activation(neg[:], d_sb[:], AT.Relu,
                                 bias=half_col[0:MCH, :], scale=-1.0)


            # T = sq on the global diagonal (column == own class), negsq off it
            tsel = sb.tile([MCH, C1], F32)
            nc.vector.tensor_tensor(tsel[:], neg[:], neg[:], OP.mult)
            nc.vector.copy_predicated(tsel[:], mask[:], sq_sb[:])

            # S_partial = (1/N^2) sum_b cnt_all[b] * sum_a cnt_own[a] * T[a, b]
            cs_ps = ps.tile([1, C1], F32, name="cs_ps")
            nc.tensor.matmul(cs_ps[:], co[:], tsel[:], start=True, stop=True)
            wrow = sb.tile([1, C1], F32)
            nc.vector.tensor_tensor(wrow[:], cs_ps[:], ca[:], OP.mult)
            acc = sb.tile([1, 1], F32)
            nc.vector.tensor_reduce(acc[:], wrow[:], AX.X, OP.add)
            part = sb.tile([1, 1], F32)
            nc.vector.tensor_scalar(part[:], acc[:], INV_N2, None, OP.mult)
            nc.gpsimd.dma_start(out_part[:], part[:])

    if not nc.is_finalized():
        nc.finalize()
    return nc


_NC_CACHE = None


def _get_nc() -> bass.Bass:
    global _NC_CACHE
    if _NC_CACHE is None:
        _NC_CACHE = build_nc()
    return _NC_CACHE


def make_in_maps(modal1_inputs, modal2_inputs, centers_param, targets):
    m1 = np.asarray(modal1_inputs, dtype=np.float32).astype(np.float16)
    m2 = np.asarray(modal2_inputs, dtype=np.float32).astype(np.float16)
    cp = np.asarray(centers_param, dtype=np.float32)
    t = np.asarray(targets).astype(np.float32).reshape(N, 1)
    cT = np.zeros((D, C1), dtype=np.float32)
    cT[:, :CR] = cp.T
    in_maps = []
    for k in range(NCORES):
        ck = k % 4
        cls0 = ck * MCH
        iocat = np.full((P, CCAT), -1.0, dtype=np.float16)
        iocat[:, :MCH] = np.arange(cls0, cls0 + MCH, dtype=np.float16)
        iocat[:, MCH:MCH + C1] = np.arange(C1, dtype=np.float16)
        in_maps.append({
            "feat": m1 if k < 4 else m2,
            "tgt": t,
            "rowcls": np.arange(cls0, cls0 + MCH,
                                dtype=np.float32).reshape(MCH, 1),
            "iocat": iocat,
            "cT": cT,
        })
    return in_maps


def run(modal1_inputs, modal2_inputs, centers_param, targets, trace=False):
    nc = _get_nc()
    in_maps = make_in_maps(modal1_inputs, modal2_inputs, centers_param, targets)
    res = run_bass_kernel_spmd(nc, in_maps, list(range(NCORES)), trace=trace)
    parts = np.array([res.results[k]["out_part"][0, 0] for k in range(NCORES)],
                     dtype=np.float32)
    loss = np.array(parts.sum(), dtype=np.float32)
    return loss, res


def kernel(modal1_inputs, modal2_inputs, centers_param, targets):
    loss, _ = run(modal1_inputs, modal2_inputs, centers_param, targets)
    return loss


# revision 27
# speedup vs baseline: 1.4701x; 1.0610x over previous
"""Cross-modal center contrastive loss on 8 Trainium2 NeuronCores.

Math: every entry of the reference's 4096x4096 distance matrix depends only on
the *class pair* (targets[i], targets[j]), because centersR[i] = class_mean[t_i]
and centers[i] = centers_param[t_i].  The loss collapses to a C x C
computation weighted by class counts:

    loss = (1/N^2) * sum_m [  sum_a cnt_a^2 * sq_m[a, a]
                            + sum_{a != b} cnt_a * cnt_b * relu(0.5 - d_m[a, b])^2 ]

with sq_m[a, b] = clip(||mean_m[a] - centers_param[b]||^2, 1e-12), d = sqrt(sq).

Device plan (SPMD over 8 cores, no collectives - the ncfw first-collective
barrier plus three serialized collective ops cost ~70us on this part, far
more than re-reading the features):
  cores 0-3 own modality 1, cores 4-7 own modality 2; core k owns the
  100-class chunk ck = k%4 (classes padded 395->400).
  Each core streams the FULL feature matrix of its modality (f16-cast on
  host, 4 MB) and accumulates, directly in PSUM:
    sums_own [100, 512]  one-hot(own-chunk) matmul against features
    cnt_cat  [1, 500]    ones-matmul against [full | own-chunk] one-hot
  The one-hot is ONE f16 DVE compare per 128-row tile against a host-built
  [iota(400) | own-class iota(100)] index tile, so the same tile feeds both
  matmuls.  Phase 2 computes the 100 x 400 distance block (means via
  inverse-count scaling, -2*dot on PE with transposed means, center norms
  folded in via a K=1 matmul), margin weighting, and a fused count-weighted
  reduction to one scalar partial.  Host sums the 8 partials.
"""

import numpy as np

from concourse import bacc, bass, mybir
from concourse import tile
from concourse.bass_utils import run_bass_kernel_spmd

F32 = mybir.dt.float32
F32R = mybir.dt.float32r
F16 = mybir.dt.float16
U8 = mybir.dt.uint8
AT = mybir.ActivationFunctionType
OP = mybir.AluOpType
AX = mybir.AxisListType

NCORES = 8
N = 4096          # batch rows
D = 512           # feature dim
CR = 395          # real number of classes
C1 = 400          # padded class count (f32r matmul wants free dim % 4 == 0)
MCH = C1 // 4     # class chunk owned per core
CCAT = 512        # one-hot width: [own 100 | full 400 | 12 pad]
P = 128           # SBUF partitions
RT = N // P       # 32 row tiles (full modality per core)
GD = 4            # row tiles per feature DMA
INV_N2 = 1.0 / (N * N)  # 2^-24, exact in f32


def build_nc() -> bass.Bass:
    nc = bacc.Bacc(None, num_devices=NCORES)

    feat = nc.dram_tensor("feat", [N, D], F16, kind="ExternalInput")
    tgt = nc.dram_tensor("tgt", [N, 1], F32, kind="ExternalInput")
    rowcls = nc.dram_tensor("rowcls", [MCH, 1], F32, kind="ExternalInput")
    iocat = nc.dram_tensor("iocat", [P, CCAT], F16, kind="ExternalInput")
    cT = nc.dram_tensor("cT", [D, C1], F32, kind="ExternalInput")
    out_part = nc.dram_tensor("out_part", [1, 1], F32, kind="ExternalOutput")

    with tile.TileContext(nc, num_cores=NCORES) as tc:
        with (
            tc.tile_pool(name="const", bufs=1) as cb,
            tc.tile_pool(name="sb", bufs=1) as sb,
            tc.tile_pool(name="stream", bufs=4) as stm,
            tc.tile_pool(name="ps", bufs=1, space="PSUM") as ps,
        ):
            # ---- constants ----
            iota = cb.tile([P, C1], F32)  # iota[p, j] = j
            nc.gpsimd.iota(iota[:], pattern=[[1, C1]], base=0,
                           channel_multiplier=0,
                           allow_small_or_imprecise_dtypes=True)
            ones_col = cb.tile([P, 1], F16)
            nc.vector.tensor_scalar(ones_col[:], iota[:, 0:1], -1.0, None,
                                    OP.is_gt)
            ones_colr = cb.tile([P, 1], F32R)
            nc.vector.tensor_scalar(ones_colr[:], iota[:, 0:1], -1.0, None,
                                    OP.is_gt)
            ones_row = cb.tile([1, P], F32R)
            nc.vector.tensor_scalar(ones_row[:], iota[0:1, 0:P], -1.0, None,
                                    OP.is_gt)
            half_col = cb.tile([P, 1], F32)
            nc.vector.memset(half_col[:], 0.5)
            ident = cb.tile([P, P], F32)
            nc.gpsimd.memset(ident[:], 0.0)
            nc.gpsimd.affine_select(out=ident[:], in_=ident[:],
                                    compare_op=OP.not_equal, fill=1.0,
                                    base=0, pattern=[[-1, P]],
                                    channel_multiplier=1)

            # warm the ACT LUTs (Square/Sqrt/Relu) while DMA streams
            warm = cb.tile([1, 1], F32)
            nc.scalar.activation(warm[:], half_col[0:1, :], AT.Square)
            nc.scalar.activation(warm[:], warm[:], AT.Sqrt)
            nc.scalar.activation(warm[:], warm[:], AT.Relu,
                                 bias=half_col[0:1, :], scale=-1.0)

            # per-core inputs
            io_sb = sb.tile([P, CCAT], F16)
            nc.sync.dma_start(io_sb[:], iocat[:])
            rc = sb.tile([MCH, 1], F32)
            nc.gpsimd.dma_start(rc[:], rowcls[:])
            tgt_sb = sb.tile([P, RT], F32)   # targets, column r = rows rP..rP+127
            nc.sync.dma_start(
                tgt_sb[:], tgt.rearrange("(r p) one -> p (r one)", r=RT))
            ct_tiles = []
            for j in range(4):
                ctj = sb.tile([P, C1], F32R, name=f"ct_{j}")
                nc.gpsimd.dma_start(ctj[:], cT[j * P:(j + 1) * P, :].bitcast(F32R))
                ct_tiles.append(ctj)

            # diagonal mask (only needs iota + rowcls; independent of phase 1)
            mask = sb.tile([MCH, C1], U8)
            nc.vector.tensor_scalar(mask[:], iota[0:MCH, :], rc[:], None,
                                    OP.is_equal)

            # PE pre-warm: the HAM clock gate needs ~3.4us of sustained
            # activity to lift PE from 1.2 to 2.4 GHz; burn idle PE time
            # before the first feature tile lands so real matmuls run warm.
            junkf = cb.tile([P, D], F16)
            nc.vector.memset(junkf[:], 0.0)
            for w in range(10):
                jp = ps.tile([1, D], F32, name="jp", tag="tp", bufs=2)
                nc.tensor.matmul(jp[:], ones_col[:], junkf[:],
                                 start=True, stop=True)

            # ---- phase 1: stream features, accumulate sums + counts ----
            sums_ps = ps.tile([MCH, D], F32, name="sums_ps")
            cnt_ps = ps.tile([1, CCAT], F32, name="cnt_ps")
            for i in range(RT // GD):
                fr = stm.tile([P, GD * D], F16, name="fr", tag="fr", bufs=3)
                nc.sync.dma_start(
                    fr[:].rearrange("p (g d) -> p g d", g=GD),
                    feat[i * GD * P:(i + 1) * GD * P, :]
                    .rearrange("(g p) d -> p g d", g=GD))
                ohs = []
                for g in range(GD):
                    r = i * GD + g
                    oh = stm.tile([P, CCAT], F16, name="oh", tag="oh", bufs=6)
                    nc.vector.tensor_scalar(oh[:], io_sb[:],
                                            tgt_sb[:, r:r + 1], None,
                                            OP.is_equal)
                    ohs.append(oh)
                    nc.tensor.matmul(sums_ps[:], oh[:, 0:MCH],
                                     fr[:, g * D:(g + 1) * D],
                                     start=(r == 0), stop=(r == RT - 1))
                for g in range(0, GD, 2):
                    r = i * GD + g
                    ohp = stm.tile([P, CCAT], F16, name="ohp", tag="ohp",
                                   bufs=3)
                    nc.vector.tensor_tensor(ohp[:], ohs[g][:], ohs[g + 1][:],
                                            OP.add)
                    nc.tensor.matmul(cnt_ps[:], ones_col[:], ohp[:],
                                     start=(r == 0), stop=(r == RT - 2))

            so = sb.tile([MCH, D], F32)
            nc.vector.tensor_copy(so[:], sums_ps[:])
            ca = sb.tile([1, C1], F32)
            nc.vector.tensor_copy(ca[:], cnt_ps[0:1, MCH:MCH + C1])
            cno_row = sb.tile([1, MCH], F32)
            nc.vector.tensor_copy(cno_row[:], cnt_ps[0:1, 0:MCH])
            # own-chunk counts as a column: PE transpose of [1, 100]
            cot_ps = ps.tile([MCH, 1], F32, name="cot_ps")
            nc.tensor.transpose(cot_ps[:], cno_row[:], ident[0:1, 0:1])
            co = sb.tile([MCH, 1], F32)
            nc.vector.tensor_copy(co[:], cot_ps[:])

            # ---- phase 2: own 100-class chunk vs all 400 centers ----
            clamped = sb.tile([MCH, 1], F32)
            nc.vector.tensor_scalar(clamped[:], co[:], 1.0, None, OP.max)
            inv = sb.tile([MCH, 1], F32)
            nc.vector.reciprocal(inv[:], clamped[:])
            inv_n2 = sb.tile([MCH, 1], F32)
            nc.vector.tensor_scalar(inv_n2[:], inv[:], -2.0, None, OP.mult)
            inv2 = sb.tile([MCH, 1], F32)
            nc.vector.tensor_tensor(inv2[:], inv[:], inv[:], OP.mult)
            snorm = sb.tile([MCH, 1], F32)
            sqtmp = sb.tile([MCH, D], F32)
            nc.scalar.activation(sqtmp[:], so[:], AT.Square, accum_out=snorm[:])
            mnorm = sb.tile([MCH, 1], F32)
            nc.vector.tensor_tensor(mnorm[:], snorm[:], inv2[:], OP.mult)
            # means scaled by -2 (so the PE dot matmul directly yields -2*dot)
            means_n2 = sb.tile([MCH, D], F32)
            nc.vector.tensor_scalar(means_n2[:], so[:], inv_n2[:], None,
                                    OP.mult)

            mt_tiles = []
            for j in range(4):
                tp = ps.tile([P, MCH], F32, name="tp", tag="tp", bufs=2)
                nc.tensor.transpose(tp[:], means_n2[:, j * P:(j + 1) * P],
                                    ident[0:MCH, 0:MCH])
                mt = sb.tile([P, MCH], F32R, name=f"mt_{j}")
                nc.vector.tensor_copy(mt[:], tp[:])
                mt_tiles.append(mt)

            cn_ps = ps.tile([1, C1], F32, name="cn_ps")
            for j in range(4):
                csq = stm.tile([P, C1], F32R, name="csq", tag="csq")
                nc.vector.tensor_tensor(csq[:], ct_tiles[j][:], ct_tiles[j][:],
                                        OP.mult)
                nc.tensor.matmul(cn_ps[:], ones_colr[:], csq[:],
                                 start=(j == 0), stop=(j == 3))
            cnorm = sb.tile([1, C1], F32R)
            nc.vector.tensor_copy(cnorm[:], cn_ps[:])

            sq_ps = ps.tile([MCH, C1], F32, name="sq_ps")
            for j in range(4):
                nc.tensor.matmul(sq_ps[:], mt_tiles[j][:], ct_tiles[j][:],
                                 start=(j == 0), stop=False)
            nc.tensor.matmul(sq_ps[:], ones_row[:, 0:MCH], cnorm[:],
                             start=False, stop=True)

            # sq = max(-2*dot + cnorm + mnorm, 1e-12)
            sq_sb = sb.tile([MCH, C1], F32)
            nc.vector.tensor_scalar(sq_sb[:], sq_ps[:], mnorm[:], 1e-12,
                                    OP.add, OP.max)
            d_sb = sb.tile([MCH, C1], F32)
            nc.scalar.activation(d_sb[:], sq_sb[:], AT.Sqrt)
            neg = sb.tile([MCH, C1], F32)
            nc.scalar.activation(neg[:], d_sb[:], AT.Relu,
                                 bias=half_col[0:MCH, :], scale=-1.0)


            # T = sq on the global diagonal (column == own class), negsq off it
            tsel = sb.tile([MCH, C1], F32)
            nc.vector.tensor_tensor(tsel[:], neg[:], neg[:], OP.mult)
            nc.vector.copy_predicated(tsel[:], mask[:], sq_sb[:])

            # S_partial = (1/N^2) sum_b cnt_all[b] * sum_a cnt_own[a] * T[a, b]
            cs_ps = ps.tile([1, C1], F32, name="cs_ps")
            nc.tensor.matmul(cs_ps[:], co[:], tsel[:], start=True, stop=True)
            wrow = sb.tile([1, C1], F32)
            nc.vector.tensor_tensor(wrow[:], cs_ps[:], ca[:], OP.mult)
            acc = sb.tile([1, 1], F32)
            nc.vector.tensor_reduce(acc[:], wrow[:], AX.X, OP.add)
            part = sb.tile([1, 1], F32)
            nc.vector.tensor_scalar(part[:], acc[:], INV_N2, None, OP.mult)
            nc.gpsimd.dma_start(out_part[:], part[:])

    if not nc.is_finalized():
        nc.finalize()
    return nc


_NC_CACHE = None


def _get_nc() -> bass.Bass:
    global _NC_CACHE
    if _NC_CACHE is None:
        _NC_CACHE = build_nc()
    return _NC_CACHE


def make_in_maps(modal1_inputs, modal2_inputs, centers_param, targets):
    m1 = np.asarray(modal1_inputs, dtype=np.float32).astype(np.float16)
    m2 = np.asarray(modal2_inputs, dtype=np.float32).astype(np.float16)
    cp = np.asarray(centers_param, dtype=np.float32)
    t = np.asarray(targets).astype(np.float32).reshape(N, 1)
    cT = np.zeros((D, C1), dtype=np.float32)
    cT[:, :CR] = cp.T
    in_maps = []
    for k in range(NCORES):
        ck = k % 4
        cls0 = ck * MCH
        iocat = np.full((P, CCAT), -1.0, dtype=np.float16)
        iocat[:, :MCH] = np.arange(cls0, cls0 + MCH, dtype=np.float16)
        iocat[:, MCH:MCH + C1] = np.arange(C1, dtype=np.float16)
        in_maps.append({
            "feat": m1 if k < 4 else m2,
            "tgt": t,
            "rowcls": np.arange(cls0, cls0 + MCH,
                                dtype=np.float32).reshape(MCH, 1),
            "iocat": iocat,
            "cT": cT,
        })
    return in_maps


def run(modal1_inputs, modal2_inputs, centers_param, targets, trace=False):
    nc = _get_nc()
    in_maps = make_in_maps(modal1_inputs, modal2_inputs, centers_param, targets)
    res = run_bass_kernel_spmd(nc, in_maps, list(range(NCORES)), trace=trace)
    parts = np.array([res.results[k]["out_part"][0, 0] for k in range(NCORES)],
                     dtype=np.float32)
    loss = np.array(parts.sum(), dtype=np.float32)
    return loss, res


def kernel(modal1_inputs, modal2_inputs, centers_param, targets):
    loss, _ = run(modal1_inputs, modal2_inputs, centers_param, targets)
    return loss
